# revision 1
# baseline (speedup 1.0000x reference)
"""NT-Xent loss on 8 Trainium2 cores (v4: cyclic 3-block symmetry, 75% exp work).

Math: with row-normalized views zjn, zin and r = [zjn; zin],
S = r@r.T / T, pos_i = (zjn_i . zin_i)/T, the kept logits for row i are
the same-view off-diagonal entries plus pos_i.  All cosine logits are
<= 1/T = 10, so with the fixed shift 10:

  lse_i  = 10 + ln( rowsum_i + epos_i )
  loss   = mean(lse_i - pos_i)

where rowsum_i = sum_{j != i} exp(S_same[i,j] - 10) and
epos_i = exp(pos_i - 10).

Symmetry: each view's 4096x4096 Gram is split into 4x4 blocks of
1024.  Core (v,s) computes its row-slab against column blocks
{s, s+1, s+2} (cyclic), i.e. 3072 of 4096 columns.  The missing block
(s, s+3) equals block (s+3, s).T, which core (v, s+3) computes as its
"+1" block; its COLUMN sums are that block's row sums.  So each core
also accumulates the column sums of its +1 block (DVE adds into a
[128,1024] fp32 accumulator; host finishes the partition reduction).

Device per tile t (hB first so the accum-ACT ends the chain):
  3 DoubleRow fp8 MMs -> psB[128,1536]; ACT exp -> scB;
  DVE row-reduce scB -> acc[:,t,0]; DVE colacc += scB[:,512:1536];
  3 MMs -> psA; ACT exp with accum_out=acc[:,t,1] (out to scrap).
Two garbage DoubleRow warm-up MMs run first to take the PE out of its
cold state.  Host: rowsum(v,s) = slabsum + colsum from core (v,s-1)
- exact fp8 diagonal; then the O(N*D) rest (normalize, pos, log, mean).
"""

import numpy as np
import ml_dtypes

N = 4096
D = 256
TEMP = 0.1
NCORES = 8
RPC = 2 * N // NCORES          # 1024 rows per core
IT = RPC // 128                # 8 i-tiles of 128 rows
W = 3 * RPC                    # 3072 columns per core (3 cyclic blocks)
HALFW = W // 2                 # 1536 cols per PSUM buffer / ACT op
NCH = HALFW // 512             # 3 column chunks per half
SC = 16.0                      # fp8 prescale (power of 2, exact)
ASCALE = (1.0 / TEMP) / (SC * SC)   # 10/256 applied in ACT

_CACHE = {}


def _build_program():
    if "nc" in _CACHE:
        return _CACHE["nc"]

    import concourse.bass as bass
    import concourse.tile as tile
    from concourse import bacc, mybir

    F8 = mybir.dt.float8e4
    BF16 = mybir.dt.bfloat16
    F32 = mybir.dt.float32

    nc = bacc.Bacc(
        "TRN2", target_bir_lowering=False, debug=False, num_devices=NCORES
    )

    # anT[h][c][p][k][col] = cols[h*1536 + c*512 + col, k*128 + p]
    anT_d = nc.dram_tensor("anT", [2, NCH, 128, 2, 512], F8, kind="ExternalInput")
    # qnT[p][k][r] = q8slab[r, k*128 + p]
    qnT_d = nc.dram_tensor("qnT", [128, 2, RPC], F8, kind="ExternalInput")
    acc_d = nc.dram_tensor("acc", [128, IT, 2], F32, kind="ExternalOutput")
    col_d = nc.dram_tensor("colacc", [128, RPC], F32, kind="ExternalOutput")

    with tile.TileContext(nc) as tc:
        with (
            tc.tile_pool(name="weights", bufs=1) as wpool,
            tc.tile_pool(name="scratch", bufs=2) as spool,
            tc.tile_pool(name="psum", bufs=2, space="PSUM") as ppool,
            tc.tile_pool(name="pwarm", bufs=1, space="PSUM") as wppool,
        ):
            qnT = wpool.tile([128, 2, RPC], F8)
            an = [
                [wpool.tile([128, 2, 512], F8, name=f"an{h}_{c}") for c in range(NCH)]
                for h in range(2)
            ]
            # DMA transfers serialize per queue, and gpsimd issues its
            # first DMA ~0.7us later than sync/scalar.  The two t0-MM
            # gates (qnT's first 128 cols, an[1][0]) go FIRST on the two
            # early queues; qnT is split so t0 waits on 32KB, not 256KB.
            nc.sync.dma_start(out=qnT[:, :, 0:128], in_=qnT_d[:, :, 0:128])
            nc.scalar.dma_start(out=an[1][0][:], in_=anT_d[1, 0])
            nc.gpsimd.dma_start(out=an[1][1][:], in_=anT_d[1, 1])
            nc.sync.dma_start(out=qnT[:, :, 128:RPC], in_=qnT_d[:, :, 128:RPC])
            nc.scalar.dma_start(out=an[1][2][:], in_=anT_d[1, 2])
            nc.gpsimd.dma_start(out=an[0][0][:], in_=anT_d[0, 0])
            nc.sync.dma_start(out=an[0][1][:], in_=anT_d[0, 1])
            nc.scalar.dma_start(out=an[0][2][:], in_=anT_d[0, 2])

            acc = wpool.tile([128, IT, 2], F32)
            colacc = wpool.tile([128, RPC], F32)
            scrap = wpool.tile([128, HALFW], BF16)
            bias = wpool.tile([128, 1], F32)
            warm = wpool.tile([128, 2, 128], F8)
            nc.vector.memset(bias[:], -1.0 / TEMP)
            nc.vector.memset(colacc[:], 0.0)
            nc.vector.memset(warm[:], 0.0)

            # two garbage DoubleRow MMs to take PE out of its cold state
            psw = wppool.tile([128, 128], F32)
            with tc.high_priority():
                for _ in range(2):
                    nc.tensor.matmul(
                        psw[:],
                        warm[:],
                        warm[:],
                        start=True,
                        stop=True,
                        perf_mode=mybir.MatmulPerfMode.DoubleRow,
                    )

            for t in range(IT):
                lhsT = qnT[:, :, t * 128:(t + 1) * 128]

                # ---- hB half: plain ACT; DVE does rowsum + colacc add
                psB = ppool.tile([128, HALFW], F32, tag="ps")
                for c in range(NCH):
                    nc.tensor.matmul(
                        psB[:, c * 512:(c + 1) * 512],
                        lhsT,
                        an[1][c][:],
                        start=True,
                        stop=True,
                        perf_mode=mybir.MatmulPerfMode.DoubleRow,
                    )
                scB = spool.tile([128, HALFW], BF16)
                last = t == IT - 1
                # last tile: ACT accumulator does the hB rowsum so the
                # final colacc stt (and its output DMA) isn't stuck
                # behind a trailing DVE reduce
                nc.scalar.activation(
                    scB[:],
                    psB[:],
                    mybir.ActivationFunctionType.Exp,
                    bias=bias[:],
                    scale=ASCALE,
                    accum_out=acc[:, t, 0:1] if last else None,
                )
                nc.vector.scalar_tensor_tensor(
                    colacc[:],
                    scB[:, 512:HALFW],
                    1.0,
                    colacc[:],
                    op0=mybir.AluOpType.bypass,
                    op1=mybir.AluOpType.add,
                )
                if not last:
                    nc.vector.tensor_reduce(
                        acc[:, t, 0:1],
                        scB[:],
                        axis=mybir.AxisListType.X,
                        op=mybir.AluOpType.add,
                    )

                # ---- hA half: ACT accumulator does the row sum
                psA = ppool.tile([128, HALFW], F32, tag="ps")
                for c in range(NCH):
                    nc.tensor.matmul(
                        psA[:, c * 512:(c + 1) * 512],
                        lhsT,
                        an[0][c][:],
                        start=True,
                        stop=True,
                        perf_mode=mybir.MatmulPerfMode.DoubleRow,
                    )
                nc.scalar.activation(
                    scrap[:],
                    psA[:],
                    mybir.ActivationFunctionType.Exp,
                    bias=bias[:],
                    scale=ASCALE,
                    accum_out=acc[:, t, 1:2],
                )

            # tiles 0..6 of acc are final once t6's RA lands; only the
            # last slice waits for the end of the ACT chain
            nc.sync.dma_start(out=acc_d[:, 0:IT - 1], in_=acc[:, 0:IT - 1])
            nc.gpsimd.dma_start(out=col_d[:], in_=colacc[:])
            nc.sync.dma_start(out=acc_d[:, IT - 1:IT], in_=acc[:, IT - 1:IT])

    nc.compile()
    _CACHE["nc"] = nc
    return nc


def _prep_inputs(z_i, z_j):
    f8 = ml_dtypes.float8_e4m3
    zin = z_i / np.sqrt(np.sum(z_i * z_i, axis=1, keepdims=True))
    zjn = z_j / np.sqrt(np.sum(z_j * z_j, axis=1, keepdims=True))
    posn = np.sum(zin * zjn, axis=1, dtype=np.float64) / TEMP      # [4096]

    q8 = [(SC * zjn).astype(f8), (SC * zin).astype(f8)]
    # exact squared norms of the quantized rows: the device Gram diagonal
    dsq = [np.sum(b.astype(np.float64) ** 2, axis=1) for b in q8]

    in_maps = []
    for c in range(NCORES):
        v, s = divmod(c, NCORES // 2)
        b = q8[v]
        brot = np.roll(b, -s * RPC, axis=0)
        # column order: [own block | +2 block | +1 block]; +1 sits in
        # hB at local cols 512:1536 so ONE colacc slice covers it
        cols = np.concatenate(
            [brot[0:RPC], brot[2 * RPC:3 * RPC], brot[RPC:2 * RPC]], axis=0
        )                                               # [3072, 256]
        anT = np.ascontiguousarray(
            cols.T.reshape(2, 128, 2, NCH, 512).transpose(2, 3, 1, 0, 4)
        )
        slab = b[s * RPC:(s + 1) * RPC]
        qnT = np.ascontiguousarray(slab.T.reshape(2, 128, RPC).transpose(1, 0, 2))
        in_maps.append({"anT": anT, "qnT": qnT})
    return in_maps, posn, dsq


def kernel(z_i, z_j):
    z_i = np.asarray(z_i, dtype=np.float32)
    z_j = np.asarray(z_j, dtype=np.float32)

    from concourse.bass_utils import run_bass_kernel_spmd

    nc = _build_program()
    in_maps, posn, dsq = _prep_inputs(z_i, z_j)

    res = run_bass_kernel_spmd(nc, in_maps, list(range(NCORES)))
    _CACHE["last_results"] = res

    nv = NCORES // 2
    rowsum = np.empty(2 * N, dtype=np.float64)
    colsum = np.empty((2, nv, RPC), dtype=np.float64)
    for c in range(NCORES):
        v, s = divmod(c, nv)
        a = res.results[c]["acc"].astype(np.float64)   # [128, IT, 2]
        rowsum[c * RPC:(c + 1) * RPC] = a.sum(axis=2).T.reshape(-1)
        colsum[v, s] = res.results[c]["colacc"].astype(np.float64).sum(axis=0)
    for v in range(2):
        for s in range(nv):
            # slab s's missing (s, s+3) block rowsums = colsums of the
            # +1 block computed by core (v, s-1)
            g0 = v * N + s * RPC
            rowsum[g0:g0 + RPC] += colsum[v, (s - 1) % nv]

    dsq_g = np.concatenate(dsq)                        # [8192] |q8 row|^2
    rowsum -= np.exp(dsq_g * ASCALE - 1.0 / TEMP)      # exact diagonal removal

    posn_g = np.concatenate([posn, posn])
    epos_g = np.exp(posn_g - 1.0 / TEMP)

    lse = 1.0 / TEMP + np.log(rowsum + epos_g)
    loss = np.mean(lse - posn_g)
    return np.array(loss, dtype=np.float32)

